# revision 1
# baseline (speedup 1.0000x reference)
"""Trainium2 Bass kernel for nn_MihGNNEmbeddingTest3 (gnn_message_passing).

Reference math:
    H = mlp(A_s @ emb)          (mlp = 3 linear layers, no activations)
    out[e] = relu(|<H[src_e], H[dst_e]>| / (||H[src_e]|| ||H[dst_e]||))

Since the mlp is affine, fold it:  H = A_s @ (emb @ W_eff^T) + b_eff.
Device work per core (node-sharded):  H_c = A_s[rows_c] @ E2 + b_eff
(E2 = emb @ W_eff^T precomputed on host), AllGather H, then per-edge
indirect row gathers + fused dot/norm reductions.

Sharding: A_s rows (and nodes) split 1024/core across 8 cores; edges
split 1024/core. A_s shard is shipped pre-transposed in bf16 so k-tiles
land directly as matmul lhsT weights.
"""

import os
import sys

import numpy as np

try:
    import concourse.bass  # noqa: F401
except ImportError:  # pragma: no cover - grading env should have PYTHONPATH set
    for p in ("/opt/trn_rl_repo", "/root/.axon_site/_ro/trn_rl_repo"):
        if os.path.isdir(p) and p not in sys.path:
            sys.path.insert(0, p)

import ml_dtypes

N, D, B = 8192, 256, 8192
N_CORES = 8
ROWS = N // N_CORES  # A_s rows / nodes per core
EPC = B // N_CORES   # edges per core
KT = N // 128        # contraction tiles
MT = ROWS // 128     # output row tiles per core
JT = EPC // 128      # edge blocks per core

_CACHE = {}
LAST_RESULTS = None  # BassKernelResults of the most recent run (for test.py)


def _build():
    import concourse.bacc as bacc
    import concourse.bass as bass
    import concourse.mybir as mybir
    import concourse.tile as tile

    fp32 = mybir.dt.float32
    bf16 = mybir.dt.bfloat16

    nc = bacc.Bacc(num_devices=N_CORES)
    # partition-major layouts: [p, k_tile, cols] so each DMA chunk reads
    # large contiguous per-partition spans from DRAM; at split in row halves
    # so m-group A's data arrives first
    ata = nc.declare_dram_parameter("ata", [128, KT, ROWS // 2], bf16, isOutput=False)
    atb = nc.declare_dram_parameter("atb", [128, KT, ROWS // 2], bf16, isOutput=False)
    e2 = nc.declare_dram_parameter("e2", [128, KT, D], bf16, isOutput=False)
    bias = nc.declare_dram_parameter("bias", [128, D], fp32, isOutput=False)
    sidx = nc.declare_dram_parameter("sidx", [128, JT], mybir.dt.int32, isOutput=False)
    didx = nc.declare_dram_parameter("didx", [128, JT], mybir.dt.int32, isOutput=False)
    out = nc.declare_dram_parameter("out", [128, JT], fp32, isOutput=True)

    with tile.TileContext(nc) as tc:
        with (
            tc.tile_pool(name="atp", bufs=1) as atp,
            tc.tile_pool(name="e2p", bufs=1) as e2p,
            tc.tile_pool(name="psum", bufs=MT, space="PSUM") as psum,
            tc.tile_pool(name="hsb", bufs=4) as hsbp,
            tc.tile_pool(name="dram", bufs=1, space="DRAM") as dram,
            tc.tile_pool(name="const", bufs=1) as constp,
            tc.tile_pool(name="gat", bufs=1) as gat,
            tc.tile_pool(name="small", bufs=1) as small,
        ):
            h_shard_a = dram.tile([ROWS // 2, D], bf16)
            h_shard_b = dram.tile([ROWS // 2, D], bf16)
            h_shards = [h_shard_a, h_shard_b]
            h_full = dram.tile([N, D], bf16)

            # Batched loads: few big DMAs with 8-16KB contiguous descriptors
            # (per-dma_start issue overhead was pacing the whole matmul).
            # Small leading chunks so the first matmuls start early; chunks
            # alternate between the two HWDGE issue engines (sync/scalar).
            AT_BOUNDS = [0, 2, 8, 16, 24, 32, 40, 48, 56, 64]
            E2_BOUNDS = [0, 4, 16, 32, 48, 64]
            at_t = [[None] * KT, [None] * KT]  # [group][k]
            e2_t = [None] * KT
            def _dma(out, in_):
                nc.sync.dma_start(out=out, in_=in_)

            def load_e2(ci):
                lo, hi = E2_BOUNDS[ci], E2_BOUNDS[ci + 1]
                ec = e2p.tile([128, hi - lo, D], bf16, name=f"e2c_{ci}", tag=f"e2c{ci}")
                _dma(ec[:], e2[:, lo:hi, :])
                for k in range(lo, hi):
                    e2_t[k] = ec[:, k - lo, :]

            def load_at(ci, g):
                src = ata if g == 0 else atb
                lo, hi = AT_BOUNDS[ci], AT_BOUNDS[ci + 1]
                ac = atp.tile(
                    [128, hi - lo, ROWS // 2], bf16,
                    name=f"atc_{g}_{ci}", tag=f"atc{g}_{ci}",
                )
                _dma(ac[:], src[:, lo:hi, :])
                for k in range(lo, hi):
                    at_t[g][k] = ac[:, k - lo, :]

            # group A (row half 0) + e2 first, then group B
            load_e2(0)
            load_at(0, 0)
            load_at(1, 0)
            load_e2(1)
            load_at(2, 0)
            load_at(3, 0)
            load_e2(2)
            load_at(4, 0)
            load_at(5, 0)
            load_e2(3)
            load_at(6, 0)
            load_at(7, 0)
            load_e2(4)
            load_at(8, 0)
            for ci in range(9):
                load_at(ci, 1)

            bias_sb = constp.tile([128, D], fp32)
            nc.sync.dma_start(out=bias_sb[:], in_=bias[:])
            sidx_sb = constp.tile([128, JT], mybir.dt.int32)
            nc.sync.dma_start(out=sidx_sb[:], in_=sidx[:])
            didx_sb = constp.tile([128, JT], mybir.dt.int32)
            nc.sync.dma_start(out=didx_sb[:], in_=didx[:])
            out_sb = constp.tile([128, JT], fp32)

            # 2 m-groups; each group's AllGather chunk overlaps the next
            # group's matmuls (and absorbs cross-core arrival skew early).
            HG = MT // 2  # m-tiles per group
            with nc.named_scope("matmul"):
                ps_t = [
                    psum.tile([128, D], fp32, name=f"ps_{m}", tag="ps")
                    for m in range(MT)
                ]
                for g in range(2):
                    ms = range(g * HG, (g + 1) * HG)
                    for k in range(KT):
                        for m in ms:
                            lm = m - g * HG
                            nc.tensor.matmul(
                                out=ps_t[m][:],
                                lhsT=at_t[g][k][:, lm * 128:(lm + 1) * 128],
                                rhs=e2_t[k],
                                start=(k == 0),
                                stop=(k == KT - 1),
                            )
                    for m in ms:
                        hs = hsbp.tile([128, D], bf16, name=f"h_{m}", tag="h")
                        nc.vector.tensor_tensor(
                            out=hs[:], in0=ps_t[m][:], in1=bias_sb[:],
                            op=mybir.AluOpType.add,
                        )
                        lm = m - g * HG
                        nc.sync.dma_start(
                            out=h_shards[g][lm * 128:(lm + 1) * 128, :], in_=hs[:]
                        )
                    with nc.named_scope(f"allgather{g}"):
                        # chunk g: rows [g*512, (g+1)*512) of every core's
                        # shard -> h_full rows [g*4096 + core*512 ...)
                        nc.gpsimd.collective_compute(
                            "AllGather",
                            mybir.AluOpType.bypass,
                            replica_groups=[list(range(N_CORES))],
                            ins=[h_shards[g][:]],
                            outs=[h_full[g * N // 2:(g + 1) * N // 2, :]],
                        )

            with nc.named_scope("edges"):
                hs_all = gat.tile([128, JT, D], bf16, name="hs_all", tag="hs_all")
                hd_all = gat.tile([128, JT, D], bf16, name="hd_all", tag="hd_all")
                dot = small.tile([128, JT], fp32, name="dot", tag="dot")
                ns = small.tile([128, JT], fp32, name="ns", tag="ns")
                nd = small.tile([128, JT], fp32, name="nd", tag="nd")
                JH = JT // 2
                for h in range(2):
                    js = slice(h * JH, (h + 1) * JH)
                    for j in range(h * JH, (h + 1) * JH):
                        nc.gpsimd.indirect_dma_start(
                            out=hs_all[:, j, :],
                            out_offset=None,
                            in_=h_full[:],
                            in_offset=bass.IndirectOffsetOnAxis(
                                ap=sidx_sb[:, j:j + 1], axis=0
                            ),
                        )
                        nc.gpsimd.indirect_dma_start(
                            out=hd_all[:, j, :],
                            out_offset=None,
                            in_=h_full[:],
                            in_offset=bass.IndirectOffsetOnAxis(
                                ap=didx_sb[:, j:j + 1], axis=0
                            ),
                        )
                    prod = gat.tile([128, JH, D], fp32, name=f"prod_{h}", tag="prod")
                    sq_s = gat.tile([128, JH, D], fp32, name=f"sq_s_{h}", tag="sq_s")
                    sq_d = gat.tile([128, JH, D], fp32, name=f"sq_d_{h}", tag="sq_d")
                    nc.vector.tensor_tensor(
                        out=prod[:], in0=hs_all[:, js, :], in1=hd_all[:, js, :],
                        op=mybir.AluOpType.mult,
                    )
                    nc.vector.tensor_reduce(
                        out=dot[:, js], in_=prod[:], axis=mybir.AxisListType.X,
                        op=mybir.AluOpType.add,
                    )
                    nc.scalar.square(sq_s[:], hs_all[:, js, :])
                    nc.scalar.square(sq_d[:], hd_all[:, js, :])
                    nc.vector.tensor_reduce(
                        out=ns[:, js], in_=sq_s[:], axis=mybir.AxisListType.X,
                        op=mybir.AluOpType.add,
                    )
                    nc.vector.tensor_reduce(
                        out=nd[:, js], in_=sq_d[:], axis=mybir.AxisListType.X,
                        op=mybir.AluOpType.add,
                    )
                nsnd = small.tile([128, JT], fp32, name="nsnd", tag="nsnd")
                nc.vector.tensor_tensor(
                    out=nsnd[:], in0=ns[:], in1=nd[:], op=mybir.AluOpType.mult
                )
                st = small.tile([128, JT], fp32, name="st", tag="st")
                nc.scalar.sqrt(st[:], nsnd[:])
                inv = small.tile([128, JT], fp32, name="inv", tag="inv")
                nc.vector.reciprocal(inv[:], st[:])
                ad = small.tile([128, JT], fp32, name="ad", tag="ad")
                nc.vector.tensor_scalar(
                    out=ad[:].bitcast(mybir.dt.uint32),
                    in0=dot[:].bitcast(mybir.dt.uint32),
                    scalar1=0x7FFFFFFF, scalar2=None,
                    op0=mybir.AluOpType.bitwise_and,
                )
                nc.vector.tensor_tensor(
                    out=out_sb[:],
                    in0=ad[:],
                    in1=inv[:],
                    op=mybir.AluOpType.mult,
                )

            nc.sync.dma_start(out=out[:], in_=out_sb[:])

    nc.compile()
    return nc


def _get_nc():
    if "nc" not in _CACHE:
        _CACHE["nc"] = _build()
    return _CACHE["nc"]


def kernel(edges, A_s, emb, Ws, bs):
    global LAST_RESULTS
    from concourse.bass_utils import run_bass_kernel_spmd

    bf16 = ml_dtypes.bfloat16
    A = np.asarray(A_s, dtype=np.float32)
    E = np.asarray(emb, dtype=np.float32)
    W = np.asarray(Ws, dtype=np.float32)
    b = np.asarray(bs, dtype=np.float32)
    ed = np.asarray(edges)

    M = W[0].T @ W[1].T @ W[2].T                      # [D, D]
    # partition-major: [128(p), KT(t), D] with row t*128+p at [p, t, :]
    E2 = np.ascontiguousarray(
        (E @ M).astype(bf16).reshape(KT, 128, D).transpose(1, 0, 2)
    )
    b_eff = (b[0] @ W[1].T + b[1]) @ W[2].T + b[2]    # [D]
    bias_rep = np.ascontiguousarray(
        np.broadcast_to(b_eff.astype(np.float32), (128, D))
    )

    def remap(n):
        # node id -> h_full row (2-chunk AllGather layout)
        o = n // ROWS
        l = n % ROWS
        g = l // (ROWS // 2)
        return g * (N // 2) + o * (ROWS // 2) + (l % (ROWS // 2))

    in_maps = []
    for c in range(N_CORES):
        at_full = (
            A[c * ROWS:(c + 1) * ROWS, :].T.astype(bf16)  # [N, ROWS]
            .reshape(KT, 128, ROWS).transpose(1, 0, 2)    # [128, KT, ROWS]
        )
        ata_c = np.ascontiguousarray(at_full[:, :, :ROWS // 2])
        atb_c = np.ascontiguousarray(at_full[:, :, ROWS // 2:])
        e = ed[c * EPC:(c + 1) * EPC].astype(np.int64)
        sidx_c = np.ascontiguousarray(
            remap(e[:, 0]).astype(np.int32).reshape(JT, 128).T
        )
        didx_c = np.ascontiguousarray(
            remap(e[:, 1]).astype(np.int32).reshape(JT, 128).T
        )
        in_maps.append(
            {"ata": ata_c, "atb": atb_c, "e2": E2, "bias": bias_rep,
             "sidx": sidx_c, "didx": didx_c}
        )

    nc = _get_nc()
    kw = {}
    if os.environ.get("KERNEL_TRACE_KW"):
        import json
        kw = json.loads(os.environ["KERNEL_TRACE_KW"])
    res = run_bass_kernel_spmd(nc, in_maps, list(range(N_CORES)), **kw)
    LAST_RESULTS = res

    out = np.concatenate(
        [np.ascontiguousarray(res.results[c]["out"].T).reshape(-1) for c in range(N_CORES)]
    )
    return np.maximum(out, 0.0).astype(np.float32)



# revision 14
# speedup vs baseline: 1.7467x; 1.7467x over previous
"""Trainium2 Bass kernel for nn_MihGNNEmbeddingTest3 (gnn_message_passing).

Reference math:
    H = mlp(A_s @ emb)          (mlp = 3 linear layers, no activations)
    out[e] = relu(|<H[src_e], H[dst_e]>| / (||H[src_e]|| ||H[dst_e]||))

Since the mlp is affine, fold it:  H = A_s @ (emb @ W_eff^T) + b_eff
(E2 = emb @ W_eff^T precomputed on host).  cos is scale-invariant, so E2
can be globally rescaled to fit fp8 range.

Layout: edge-pre-permuted, collective-free.  Each core computes
H^T columns for exactly the 2048 endpoint nodes of its own 1024 edges
(host gathers the needed A_s rows per core), via fp8-e4m3 DoubleRow
matmuls (K=256 per pass, moving free dim 512).  A_s is shipped as the
residual A-0.5 (quantization error scales with the residual, not the
value; the mean's contribution 0.5*colsum(E2) folds into the bias).
Bias lands via the per-partition bias of the ACT copy that stages
H^T out of PSUM.  dot/||h||^2 reduce over d (the partition dim) with
data-stationary matmuls against a ones column; the final
|dot|*rsqrt(ns*nd) runs on [128, 8] tiles at full lane parallelism.

Columns per core are grouped in 4 blocks of 512 = [src 256 | dst 256]
so each block's dot/norm math reads one PSUM tile; blocks are split in
2 phases of 2 so phase-0 reductions overlap phase-1 matmuls.
"""

import os
import sys

import numpy as np

try:
    import concourse.bass  # noqa: F401
except ImportError:  # pragma: no cover - grading env should have PYTHONPATH set
    for p in ("/opt/trn_rl_repo", "/root/.axon_site/_ro/trn_rl_repo"):
        if os.path.isdir(p) and p not in sys.path:
            sys.path.insert(0, p)

import ml_dtypes

N, D, B = 8192, 256, 8192
N_CORES = 8
EPC = B // N_CORES    # edges per core
COLS = 2 * EPC        # H^T columns per core (src+dst)
KT2 = N // 256        # DoubleRow k-steps (256 contraction each)
JT = EPC // 128       # edge blocks per core
NPH = 2               # phases (2 column-blocks each)
E2_SCALE_TARGET = 200.0

_CACHE = {}
LAST_RESULTS = None  # BassKernelResults of the most recent run (for test.py)


def _build():
    import concourse.bacc as bacc
    import concourse.bass as bass  # noqa: F401
    import concourse.mybir as mybir
    import concourse.tile as tile

    fp32 = mybir.dt.float32
    bf16 = mybir.dt.bfloat16
    fp8 = mybir.dt.float8e4
    DR = mybir.MatmulPerfMode.DoubleRow

    nc = bacc.Bacc(num_devices=N_CORES)
    # a8[p, ph, t, ko, col] = Rq[node(ph*1024+col), t*256 + ko*128 + p]
    a8 = nc.declare_dram_parameter(
        "a8", [128, NPH, KT2, 2, COLS // NPH], fp8, isOutput=False
    )
    # e28[p, t, ko, d] = E2q[t*256 + ko*128 + p, d]
    e28 = nc.declare_dram_parameter("e28", [128, KT2, 2, D], fp8, isOutput=False)
    bias2 = nc.declare_dram_parameter("bias2", [128, 2], fp32, isOutput=False)
    out = nc.declare_dram_parameter("out", [128, JT], fp32, isOutput=True)

    A_BOUNDS = [0, 2, 6, 10, 14, 18, 22, 26, 32]
    E_BOUNDS = [0, 4, 12, 20, 32]

    with tile.TileContext(nc) as tc:
        with (
            tc.tile_pool(name="ap", bufs=1) as apool,
            tc.tile_pool(name="ep", bufs=1) as epool,
            tc.tile_pool(name="psum", bufs=8, space="PSUM") as psum,
            tc.tile_pool(name="stage", bufs=12) as stage,
            tc.tile_pool(name="const", bufs=1) as constp,
        ):
            a_t = [[None] * KT2 for _ in range(NPH)]
            e_t = [None] * KT2

            def load_a(ph, ci):
                lo, hi = A_BOUNDS[ci], A_BOUNDS[ci + 1]
                ac = apool.tile(
                    [128, hi - lo, 2, COLS // NPH], fp8,
                    name=f"ac_{ph}_{ci}", tag=f"ac{ph}_{ci}",
                )
                nc.sync.dma_start(out=ac[:], in_=a8[:, ph, lo:hi, :, :])
                for t in range(lo, hi):
                    a_t[ph][t] = ac[:, t - lo, :, :]

            def load_e(ci):
                lo, hi = E_BOUNDS[ci], E_BOUNDS[ci + 1]
                ec = epool.tile(
                    [128, hi - lo, 2, D], fp8, name=f"ec_{ci}", tag=f"ec{ci}"
                )
                nc.sync.dma_start(out=ec[:], in_=e28[:, lo:hi, :, :])
                for t in range(lo, hi):
                    e_t[t] = ec[:, t - lo, :, :]

            load_e(0)
            load_a(0, 0)
            load_a(0, 1)
            load_e(1)
            load_a(0, 2)
            load_a(0, 3)
            load_e(2)
            load_a(0, 4)
            load_a(0, 5)
            load_e(3)
            load_a(0, 6)
            load_a(0, 7)
            for ci in range(8):
                load_a(1, ci)

            bias_sb = constp.tile([128, 2], fp32)
            nc.sync.dma_start(out=bias_sb[:], in_=bias2[:])
            ones1 = constp.tile([128, 1], bf16)
            nc.vector.memset(ones1[:], 1.0)
            out_sb = constp.tile([128, JT], fp32)

            # main matmuls + per-block reductions
            red = None  # [128, 24] used: dot j | ns 8+j | nd 16+j
            ps_t = [[None, None] for _ in range(4)]  # [cb][dtile]

            def reduce_cb(cb):
                nonlocal red
                if red is None:
                    # same tag as ps: cycles within the 8-buffer pool (the
                    # 9th allocation reuses a released phase-0 bank)
                    red = psum.tile([128, 512], fp32, name="red", tag="ps")
                tiles = []
                for dt in range(2):
                    ps = ps_t[cb][dt]
                    hs_sb = stage.tile(
                        [128, 256], fp32, name=f"hs_{cb}_{dt}", tag="hs"
                    )
                    nc.scalar.activation(
                        hs_sb[:], ps[:, 0:256],
                        mybir.ActivationFunctionType.Identity,
                        bias=bias_sb[:, dt:dt + 1],
                    )
                    hd_sb = stage.tile(
                        [128, 256], fp32, name=f"hd_{cb}_{dt}", tag="hd"
                    )
                    nc.scalar.activation(
                        hd_sb[:], ps[:, 256:512],
                        mybir.ActivationFunctionType.Identity,
                        bias=bias_sb[:, dt:dt + 1],
                    )
                    P = stage.tile([128, 256], bf16, name=f"P_{cb}_{dt}", tag="P")
                    nc.vector.tensor_tensor(
                        out=P[:], in0=hs_sb[:], in1=hd_sb[:],
                        op=mybir.AluOpType.mult,
                    )
                    S2 = stage.tile([128, 256], bf16, name=f"S2_{cb}_{dt}", tag="S2")
                    nc.vector.tensor_tensor(
                        out=S2[:], in0=hs_sb[:], in1=hs_sb[:],
                        op=mybir.AluOpType.mult,
                    )
                    D2 = stage.tile([128, 256], bf16, name=f"D2_{cb}_{dt}", tag="D2")
                    nc.scalar.square(D2[:], hd_sb[:])
                    tiles.append((P, S2, D2))
                # dt pair adjacent per accumulation group: a group-start in
                # the same PSUM bank between a pair would reset its state
                for c in range(2):
                    j = cb * 2 + c
                    for q in range(3):
                        for dt in range(2):
                            nc.tensor.matmul(
                                out=red[:, 8 * q + j:8 * q + j + 1],
                                lhsT=tiles[dt][q][:, c * 128:(c + 1) * 128],
                                rhs=ones1[:],
                                start=(dt == 0),
                                stop=(dt == 1),
                            )

            with nc.named_scope("matmul"):
                for ph in range(NPH):
                    for cb2 in range(2):
                        for dt in range(2):
                            ps_t[ph * 2 + cb2][dt] = psum.tile(
                                [128, 512], fp32,
                                name=f"ps_{ph}_{cb2}_{dt}", tag="ps",
                            )
                    for t in range(KT2):
                        for dt in range(2):
                            for cb2 in range(2):
                                nc.tensor.matmul(
                                    out=ps_t[ph * 2 + cb2][dt][:],
                                    lhsT=e_t[t][:, :, dt * 128:(dt + 1) * 128],
                                    rhs=a_t[ph][t][:, :, cb2 * 512:(cb2 + 1) * 512],
                                    start=(t == 0),
                                    stop=(t == KT2 - 1),
                                    perf_mode=DR,
                                )
                    with nc.named_scope(f"reduce{ph}"):
                        reduce_cb(ph * 2)
                        reduce_cb(ph * 2 + 1)

            with nc.named_scope("final"):
                red_sb = constp.tile([128, 24], fp32)
                nc.vector.tensor_copy(red_sb[:], red[:, 0:24])
                nsnd = constp.tile([128, JT], fp32)
                nc.vector.tensor_tensor(
                    out=nsnd[:], in0=red_sb[:, 8:16], in1=red_sb[:, 16:24],
                    op=mybir.AluOpType.mult,
                )
                st = constp.tile([128, JT], fp32)
                nc.scalar.sqrt(st[:], nsnd[:])
                rinv = constp.tile([128, JT], fp32)
                nc.vector.reciprocal(rinv[:], st[:])
                ad = constp.tile([128, JT], fp32)
                nc.vector.tensor_scalar(
                    out=ad[:].bitcast(mybir.dt.uint32),
                    in0=red_sb[:, 0:8].bitcast(mybir.dt.uint32),
                    scalar1=0x7FFFFFFF, scalar2=None,
                    op0=mybir.AluOpType.bitwise_and,
                )
                nc.vector.tensor_tensor(
                    out=out_sb[:], in0=ad[:], in1=rinv[:],
                    op=mybir.AluOpType.mult,
                )
                nc.sync.dma_start(out=out[:], in_=out_sb[:])

    nc.compile()
    return nc


def _get_nc():
    if "nc" not in _CACHE:
        _CACHE["nc"] = _build()
    return _CACHE["nc"]


def kernel(edges, A_s, emb, Ws, bs):
    global LAST_RESULTS
    from concourse.bass_utils import run_bass_kernel_spmd

    e4 = ml_dtypes.float8_e4m3fn
    A = np.asarray(A_s, dtype=np.float32)
    E = np.asarray(emb, dtype=np.float32)
    W = np.asarray(Ws, dtype=np.float32)
    b = np.asarray(bs, dtype=np.float32)
    ed = np.asarray(edges)

    M = W[0].T @ W[1].T @ W[2].T                      # [D, D]
    E2 = E @ M                                        # [N, D]
    b_eff = (b[0] @ W[1].T + b[1]) @ W[2].T + b[2]    # [D]
    s = E2_SCALE_TARGET / np.abs(E2).max()
    E2s = E2 * s
    bias_tot = (0.5 * E2.sum(axis=0) + b_eff) * s     # mean part + bias
    bias2 = np.ascontiguousarray(bias_tot.reshape(2, 128).T.astype(np.float32))

    # e28[p, t, ko, d] = E2q[t*256 + ko*128 + p, d]
    E2q = np.clip(E2s, -240.0, 240.0).astype(e4)
    e28 = np.ascontiguousarray(
        E2q.reshape(KT2, 2, 128, D).transpose(2, 0, 1, 3)
    )

    # residual of A in fp8 (mean folded into bias above)
    Rq = (A - 0.5).astype(e4)                         # [N, N]

    in_maps = []
    for c in range(N_CORES):
        e = ed[c * EPC:(c + 1) * EPC].astype(np.int64)
        src, dst = e[:, 0], e[:, 1]
        a8 = np.empty((128, NPH, KT2, 2, COLS // NPH), dtype=e4)
        for ph in range(NPH):
            nodes = []
            for cb2 in range(2):
                cb = ph * 2 + cb2
                sl = slice(cb * 256, (cb + 1) * 256)
                nodes.append(src[sl])
                nodes.append(dst[sl])
            nodes = np.concatenate(nodes)             # [1024]
            Rga = Rq[nodes]                           # [1024, 8192] fp8
            # -> [ki, t, ko, col]
            a8[:, ph] = Rga.T.reshape(KT2, 2, 128, COLS // NPH).transpose(2, 0, 1, 3)
        in_maps.append({"a8": a8, "e28": e28, "bias2": bias2})

    nc = _get_nc()
    kw = {}
    if os.environ.get("KERNEL_TRACE_KW"):
        import json
        kw = json.loads(os.environ["KERNEL_TRACE_KW"])
    res = run_bass_kernel_spmd(nc, in_maps, list(range(N_CORES)), **kw)
    LAST_RESULTS = res

    out = np.concatenate(
        [np.ascontiguousarray(res.results[c]["out"].T).reshape(-1)
         for c in range(N_CORES)]
    )
    return np.maximum(out, 0.0).astype(np.float32)


# revision 15
# speedup vs baseline: 1.9866x; 1.1373x over previous
"""Trainium2 Bass kernel for nn_MihGNNEmbeddingTest3 (gnn_message_passing).

Reference math:
    H = mlp(A_s @ emb)          (mlp = 3 linear layers, no activations)
    out[e] = relu(|<H[src_e], H[dst_e]>| / (||H[src_e]|| ||H[dst_e]||))

Since the mlp is affine, fold it:  H = A_s @ (emb @ W_eff^T) + b_eff
(E2 = emb @ W_eff^T precomputed on host).  cos is scale-invariant, so E2
can be globally rescaled to fit fp8 range.

Layout: edge-pre-permuted, collective-free.  Each core computes
H^T columns for exactly the 2048 endpoint nodes of its own 1024 edges
(host gathers the needed A_s rows per core), via fp8-e4m3 DoubleRow
matmuls (K=256 per pass, moving free dim 512).  A_s is shipped as the
residual A-0.5 (quantization error scales with the residual, not the
value; the mean's contribution 0.5*colsum(E2) folds into the bias).
Bias lands via the per-partition bias of the ACT copy that stages
H^T out of PSUM.  dot/||h||^2 reduce over d (the partition dim) with
data-stationary matmuls against a ones column; the final
|dot|*rsqrt(ns*nd) runs on [128, 8] tiles at full lane parallelism.

Columns per core are grouped in 4 blocks of 512 = [src 256 | dst 256]
so each block's dot/norm math reads one PSUM tile; blocks are split in
2 phases of 2 so phase-0 reductions overlap phase-1 matmuls.
"""

import os
import sys

import numpy as np

try:
    import concourse.bass  # noqa: F401
except ImportError:  # pragma: no cover - grading env should have PYTHONPATH set
    for p in ("/opt/trn_rl_repo", "/root/.axon_site/_ro/trn_rl_repo"):
        if os.path.isdir(p) and p not in sys.path:
            sys.path.insert(0, p)

import ml_dtypes

N, D, B = 8192, 256, 8192
N_CORES = 8
EPC = B // N_CORES    # edges per core
COLS = 2 * EPC        # H^T columns per core (src+dst)
KT2 = N // 256        # DoubleRow k-steps (256 contraction each)
JT = EPC // 128       # edge blocks per core
NPH = 2               # phases (2 column-blocks each)
E2_SCALE_TARGET = 200.0

_CACHE = {}
LAST_RESULTS = None  # BassKernelResults of the most recent run (for test.py)


def _build():
    import concourse.bacc as bacc
    import concourse.bass as bass  # noqa: F401
    import concourse.mybir as mybir
    import concourse.tile as tile

    fp32 = mybir.dt.float32
    bf16 = mybir.dt.bfloat16
    fp8 = mybir.dt.float8e4
    DR = mybir.MatmulPerfMode.DoubleRow

    nc = bacc.Bacc(num_devices=N_CORES)
    # a8[p, ph, t, ko, col] = Rq[node(ph*1024+col), t*256 + ko*128 + p]
    a8 = nc.declare_dram_parameter(
        "a8", [128, NPH, KT2, 2, COLS // NPH], fp8, isOutput=False
    )
    # e28[p, t, ko, d] = E2q[t*256 + ko*128 + p, d]
    e28 = nc.declare_dram_parameter("e28", [128, KT2, 2, D], fp8, isOutput=False)
    # bias pre-broadcast along the free dim so a single DVE tensor_tensor
    # does PSUM->SBUF staging + bias add (keeps ACT free for the final sqrt)
    biasb = nc.declare_dram_parameter("biasb", [128, 2, D], fp32, isOutput=False)
    out = nc.declare_dram_parameter("out", [128, JT], fp32, isOutput=True)

    A_BOUNDS = [0, 1, 3, 5, 8, 11, 14, 17, 20, 23, 26, 29, 32]
    E_BOUNDS = [0, 2, 6, 10, 14, 18, 23, 28, 32]

    with tile.TileContext(nc) as tc:
        with (
            tc.tile_pool(name="ap", bufs=1) as apool,
            tc.tile_pool(name="ep", bufs=1) as epool,
            tc.tile_pool(name="psum", bufs=8, space="PSUM") as psum,
            tc.tile_pool(name="stage", bufs=12) as stage,
            tc.tile_pool(name="const", bufs=1) as constp,
        ):
            a_t = [[None] * KT2 for _ in range(NPH)]
            e_t = [None] * KT2
            issue_eng = [nc.sync, nc.scalar]
            issue_i = [0]

            def _dma(o, i):
                issue_eng[issue_i[0] % 2].dma_start(out=o, in_=i)
                issue_i[0] += 1

            # bias first: a late bias at the back of the DMA queue stalled
            # the whole reduce (and the 9th PSUM buffer) behind 18MB
            bias_sb = constp.tile([128, 2, D], fp32)
            nc.sync.dma_start(out=bias_sb[:], in_=biasb[:])

            def load_a(ph, ci):
                lo, hi = A_BOUNDS[ci], A_BOUNDS[ci + 1]
                ac = apool.tile(
                    [128, hi - lo, 2, COLS // NPH], fp8,
                    name=f"ac_{ph}_{ci}", tag=f"ac{ph}_{ci}",
                )
                _dma(ac[:], a8[:, ph, lo:hi, :, :])
                for t in range(lo, hi):
                    a_t[ph][t] = ac[:, t - lo, :, :]

            def load_e(ci):
                lo, hi = E_BOUNDS[ci], E_BOUNDS[ci + 1]
                ec = epool.tile(
                    [128, hi - lo, 2, D], fp8, name=f"ec_{ci}", tag=f"ec{ci}"
                )
                _dma(ec[:], e28[:, lo:hi, :, :])
                for t in range(lo, hi):
                    e_t[t] = ec[:, t - lo, :, :]

            load_e(0)
            load_a(0, 0)
            load_a(0, 1)
            load_e(1)
            load_a(0, 2)
            load_e(2)
            load_a(0, 3)
            load_a(0, 4)
            load_e(3)
            load_a(0, 5)
            load_a(0, 6)
            load_e(4)
            load_a(0, 7)
            load_a(0, 8)
            load_e(5)
            load_a(0, 9)
            load_a(0, 10)
            load_e(6)
            load_a(0, 11)
            load_e(7)
            for ci in range(12):
                load_a(1, ci)

            ones1 = constp.tile([128, 1], bf16)
            nc.vector.memset(ones1[:], 1.0)
            out_sb = constp.tile([128, JT], fp32)
            # preload the ACT sqrt table while DMAs stream (else its 1.3us
            # table load lands on the critical tail)
            junk = constp.tile([128, 1], fp32)
            nc.vector.memset(junk[:], 1.0)
            junk2 = constp.tile([128, 1], fp32)
            nc.scalar.sqrt(junk2[:], junk[:])

            # main matmuls + per-block reductions
            red = None  # [128, 24] used: dot j | ns 8+j | nd 16+j
            ps_t = [[None, None] for _ in range(4)]  # [cb][dtile]

            def reduce_cb(cb):
                nonlocal red
                if red is None:
                    # same tag as ps: cycles within the 8-buffer pool (the
                    # 9th allocation reuses a released phase-0 bank)
                    red = psum.tile([128, 512], fp32, name="red", tag="ps")
                tiles = []
                for dt in range(2):
                    ps = ps_t[cb][dt]
                    hs_sb = stage.tile(
                        [128, 256], fp32, name=f"hs_{cb}_{dt}", tag="hs"
                    )
                    nc.vector.tensor_tensor(
                        out=hs_sb[:], in0=ps[:, 0:256], in1=bias_sb[:, dt, :],
                        op=mybir.AluOpType.add,
                    )
                    hd_sb = stage.tile(
                        [128, 256], fp32, name=f"hd_{cb}_{dt}", tag="hd"
                    )
                    nc.vector.tensor_tensor(
                        out=hd_sb[:], in0=ps[:, 256:512], in1=bias_sb[:, dt, :],
                        op=mybir.AluOpType.add,
                    )
                    P = stage.tile([128, 256], bf16, name=f"P_{cb}_{dt}", tag="P")
                    nc.vector.tensor_tensor(
                        out=P[:], in0=hs_sb[:], in1=hd_sb[:],
                        op=mybir.AluOpType.mult,
                    )
                    S2 = stage.tile([128, 256], bf16, name=f"S2_{cb}_{dt}", tag="S2")
                    nc.vector.tensor_tensor(
                        out=S2[:], in0=hs_sb[:], in1=hs_sb[:],
                        op=mybir.AluOpType.mult,
                    )
                    D2 = stage.tile([128, 256], bf16, name=f"D2_{cb}_{dt}", tag="D2")
                    nc.vector.tensor_tensor(
                        out=D2[:], in0=hd_sb[:], in1=hd_sb[:],
                        op=mybir.AluOpType.mult,
                    )
                    tiles.append((P, S2, D2))
                # dt pair adjacent per accumulation group: a group-start in
                # the same PSUM bank between a pair would reset its state
                for c in range(2):
                    j = cb * 2 + c
                    for q in range(3):
                        for dt in range(2):
                            nc.tensor.matmul(
                                out=red[:, 8 * q + j:8 * q + j + 1],
                                lhsT=tiles[dt][q][:, c * 128:(c + 1) * 128],
                                rhs=ones1[:],
                                start=(dt == 0),
                                stop=(dt == 1),
                            )

            with nc.named_scope("matmul"):
                for ph in range(NPH):
                    for cb2 in range(2):
                        for dt in range(2):
                            ps_t[ph * 2 + cb2][dt] = psum.tile(
                                [128, 512], fp32,
                                name=f"ps_{ph}_{cb2}_{dt}", tag="ps",
                            )
                    for t in range(KT2):
                        for dt in range(2):
                            for cb2 in range(2):
                                nc.tensor.matmul(
                                    out=ps_t[ph * 2 + cb2][dt][:],
                                    lhsT=e_t[t][:, :, dt * 128:(dt + 1) * 128],
                                    rhs=a_t[ph][t][:, :, cb2 * 512:(cb2 + 1) * 512],
                                    start=(t == 0),
                                    stop=(t == KT2 - 1),
                                    perf_mode=DR,
                                )
                    with nc.named_scope(f"reduce{ph}"):
                        reduce_cb(ph * 2)
                        reduce_cb(ph * 2 + 1)

            with nc.named_scope("final"):
                red_sb = constp.tile([128, 24], fp32)
                nc.vector.tensor_copy(red_sb[:], red[:, 0:24])
                nsnd = constp.tile([128, JT], fp32)
                nc.vector.tensor_tensor(
                    out=nsnd[:], in0=red_sb[:, 8:16], in1=red_sb[:, 16:24],
                    op=mybir.AluOpType.mult,
                )
                st = constp.tile([128, JT], fp32)
                nc.scalar.sqrt(st[:], nsnd[:])
                rinv = constp.tile([128, JT], fp32)
                nc.vector.reciprocal(rinv[:], st[:])
                ad = constp.tile([128, JT], fp32)
                nc.vector.tensor_scalar(
                    out=ad[:].bitcast(mybir.dt.uint32),
                    in0=red_sb[:, 0:8].bitcast(mybir.dt.uint32),
                    scalar1=0x7FFFFFFF, scalar2=None,
                    op0=mybir.AluOpType.bitwise_and,
                )
                nc.vector.tensor_tensor(
                    out=out_sb[:], in0=ad[:], in1=rinv[:],
                    op=mybir.AluOpType.mult,
                )
                nc.sync.dma_start(out=out[:], in_=out_sb[:])

    nc.compile()
    return nc


def _get_nc():
    if "nc" not in _CACHE:
        _CACHE["nc"] = _build()
    return _CACHE["nc"]


def kernel(edges, A_s, emb, Ws, bs):
    global LAST_RESULTS
    from concourse.bass_utils import run_bass_kernel_spmd

    e4 = ml_dtypes.float8_e4m3fn
    A = np.asarray(A_s, dtype=np.float32)
    E = np.asarray(emb, dtype=np.float32)
    W = np.asarray(Ws, dtype=np.float32)
    b = np.asarray(bs, dtype=np.float32)
    ed = np.asarray(edges)

    M = W[0].T @ W[1].T @ W[2].T                      # [D, D]
    E2 = E @ M                                        # [N, D]
    b_eff = (b[0] @ W[1].T + b[1]) @ W[2].T + b[2]    # [D]
    s = E2_SCALE_TARGET / np.abs(E2).max()
    E2s = E2 * s
    bias_tot = (0.5 * E2.sum(axis=0) + b_eff) * s     # mean part + bias
    biasb = np.ascontiguousarray(
        np.broadcast_to(
            bias_tot.reshape(2, 128).T[:, :, None].astype(np.float32),
            (128, 2, D),
        )
    )

    # e28[p, t, ko, d] = E2q[t*256 + ko*128 + p, d]
    E2q = np.clip(E2s, -240.0, 240.0).astype(e4)
    e28 = np.ascontiguousarray(
        E2q.reshape(KT2, 2, 128, D).transpose(2, 0, 1, 3)
    )

    # residual of A in fp8 (mean folded into bias above)
    Rq = (A - 0.5).astype(e4)                         # [N, N]

    in_maps = []
    for c in range(N_CORES):
        e = ed[c * EPC:(c + 1) * EPC].astype(np.int64)
        src, dst = e[:, 0], e[:, 1]
        a8 = np.empty((128, NPH, KT2, 2, COLS // NPH), dtype=e4)
        for ph in range(NPH):
            nodes = []
            for cb2 in range(2):
                cb = ph * 2 + cb2
                sl = slice(cb * 256, (cb + 1) * 256)
                nodes.append(src[sl])
                nodes.append(dst[sl])
            nodes = np.concatenate(nodes)             # [1024]
            Rga = Rq[nodes]                           # [1024, 8192] fp8
            # -> [ki, t, ko, col]
            a8[:, ph] = Rga.T.reshape(KT2, 2, 128, COLS // NPH).transpose(2, 0, 1, 3)
        in_maps.append({"a8": a8, "e28": e28, "biasb": biasb})

    nc = _get_nc()
    kw = {}
    if os.environ.get("KERNEL_TRACE_KW"):
        import json
        kw = json.loads(os.environ["KERNEL_TRACE_KW"])
    res = run_bass_kernel_spmd(nc, in_maps, list(range(N_CORES)), **kw)
    LAST_RESULTS = res

    out = np.concatenate(
        [np.ascontiguousarray(res.results[c]["out"].T).reshape(-1)
         for c in range(N_CORES)]
    )
    return np.maximum(out, 0.0).astype(np.float32)


# revision 16
# speedup vs baseline: 2.1310x; 1.0727x over previous
"""Trainium2 Bass kernel for nn_MihGNNEmbeddingTest3 (gnn_message_passing).

Reference math:
    H = mlp(A_s @ emb)          (mlp = 3 linear layers, no activations)
    out[e] = relu(|<H[src_e], H[dst_e]>| / (||H[src_e]|| ||H[dst_e]||))

Since the mlp is affine, fold it:  H = A_s @ (emb @ W_eff^T) + b_eff
(E2 = emb @ W_eff^T precomputed on host).  cos is scale-invariant, so E2
can be globally rescaled to fit fp8 range.

Layout: edge-pre-permuted, collective-free.  Each core computes
H^T columns for exactly the 2048 endpoint nodes of its own 1024 edges
(host gathers the needed A_s rows per core), via fp8-e4m3 DoubleRow
matmuls (K=256 per pass, moving free dim 512).  A_s is shipped as the
residual A-0.5 (quantization error scales with the residual, not the
value; the mean's contribution 0.5*colsum(E2) folds into the bias).
Bias lands via the per-partition bias of the ACT copy that stages
H^T out of PSUM.  dot/||h||^2 reduce over d (the partition dim) with
data-stationary matmuls against a ones column; the final
|dot|*rsqrt(ns*nd) runs on [128, 8] tiles at full lane parallelism.

Columns per core are grouped in 4 blocks of 512 = [src 256 | dst 256]
so each block's dot/norm math reads one PSUM tile; blocks are split in
2 phases of 2 so phase-0 reductions overlap phase-1 matmuls.
"""

import os
import sys

import numpy as np

try:
    import concourse.bass  # noqa: F401
except ImportError:  # pragma: no cover - grading env should have PYTHONPATH set
    for p in ("/opt/trn_rl_repo", "/root/.axon_site/_ro/trn_rl_repo"):
        if os.path.isdir(p) and p not in sys.path:
            sys.path.insert(0, p)

import ml_dtypes

N, D, B = 8192, 256, 8192
N_CORES = 8
EPC = B // N_CORES    # edges per core
COLS = 2 * EPC        # H^T columns per core (src+dst)
KT2 = N // 256        # DoubleRow k-steps (256 contraction each)
JT = EPC // 128       # edge blocks per core
NPH = 2               # phases (2 column-blocks each)
E2_SCALE_TARGET = 200.0

_CACHE = {}
LAST_RESULTS = None  # BassKernelResults of the most recent run (for test.py)


def _build():
    import concourse.bacc as bacc
    import concourse.bass as bass  # noqa: F401
    import concourse.mybir as mybir
    import concourse.tile as tile

    fp32 = mybir.dt.float32
    bf16 = mybir.dt.bfloat16
    fp8 = mybir.dt.float8e4
    DR = mybir.MatmulPerfMode.DoubleRow

    nc = bacc.Bacc(num_devices=N_CORES)
    # a8[p, cb, t, ko, col] = Rq[node(cb*512+col), t*256 + ko*128 + p]
    a8 = nc.declare_dram_parameter(
        "a8", [128, 4, KT2, 2, 512], fp8, isOutput=False
    )
    # e28[p, t, ko, d] = E2q[t*256 + ko*128 + p, d]
    e28 = nc.declare_dram_parameter("e28", [128, KT2, 2, D], fp8, isOutput=False)
    # bias pre-broadcast along the free dim so a single DVE tensor_tensor
    # does PSUM->SBUF staging + bias add (keeps ACT free for the final sqrt)
    biasb = nc.declare_dram_parameter("biasb", [128, 2, D], fp32, isOutput=False)
    out = nc.declare_dram_parameter("out", [128, JT], fp32, isOutput=True)

    AB01 = [0, 1, 3, 5, 8, 11, 14, 18, 22, 27, 32]
    AB23 = [0, 4, 8, 12, 16, 20, 26, 32]
    E_BOUNDS = [0, 2, 6, 10, 14, 18, 23, 28, 32]

    with tile.TileContext(nc) as tc:
        with (
            tc.tile_pool(name="ap", bufs=1) as apool,
            tc.tile_pool(name="ep", bufs=1) as epool,
            tc.tile_pool(name="psum", bufs=8, space="PSUM") as psum,
            tc.tile_pool(name="stage", bufs=12) as stage,
            tc.tile_pool(name="const", bufs=1) as constp,
        ):
            a_t = [[None] * KT2 for _ in range(4)]
            e_t = [None] * KT2
            issue_eng = [nc.sync, nc.scalar]
            issue_i = [0]

            def _dma(o, i):
                issue_eng[issue_i[0] % 2].dma_start(out=o, in_=i)
                issue_i[0] += 1

            # bias first (scalar ring so it doesn't delay the first a chunk):
            # a late bias at the back of the DMA queue once stalled the whole
            # reduce (and the 9th PSUM buffer) behind 18MB
            bias_sb = constp.tile([128, 2, D], fp32)
            nc.scalar.dma_start(out=bias_sb[:], in_=biasb[:])

            def load_a(cb, bounds, ci):
                lo, hi = bounds[ci], bounds[ci + 1]
                ac = apool.tile(
                    [128, hi - lo, 2, 512], fp8,
                    name=f"ac_{cb}_{ci}", tag=f"ac{cb}_{ci}",
                )
                _dma(ac[:], a8[:, cb, lo:hi, :, :])
                for t in range(lo, hi):
                    a_t[cb][t] = ac[:, t - lo, :, :]

            def load_e(ci):
                lo, hi = E_BOUNDS[ci], E_BOUNDS[ci + 1]
                ec = epool.tile(
                    [128, hi - lo, 2, D], fp8, name=f"ec_{ci}", tag=f"ec{ci}"
                )
                _dma(ec[:], e28[:, lo:hi, :, :])
                for t in range(lo, hi):
                    e_t[t] = ec[:, t - lo, :, :]

            load_e(0)
            load_a(0, AB01, 0)
            load_a(1, AB01, 0)
            load_e(1)
            load_a(0, AB01, 1)
            load_a(1, AB01, 1)
            load_e(2)
            load_a(0, AB01, 2)
            load_a(1, AB01, 2)
            load_e(3)
            load_a(0, AB01, 3)
            load_a(1, AB01, 3)
            load_e(4)
            load_a(0, AB01, 4)
            load_a(1, AB01, 4)
            load_e(5)
            load_a(0, AB01, 5)
            load_a(1, AB01, 5)
            load_e(6)
            load_a(0, AB01, 6)
            load_a(1, AB01, 6)
            load_e(7)
            for ci in range(7, 10):
                load_a(0, AB01, ci)
                load_a(1, AB01, ci)
            for ci in range(7):
                load_a(2, AB23, ci)
            for ci in range(7):
                load_a(3, AB23, ci)

            ones1 = constp.tile([128, 1], bf16)
            nc.vector.memset(ones1[:], 1.0)
            out_sb = constp.tile([128, JT], fp32)
            # preload ACT square+sqrt tables while DMAs stream (else their
            # 1.3us table loads land on the critical tail)
            junk = constp.tile([128, 1], fp32)
            nc.vector.memset(junk[:], 1.0)
            junk2 = constp.tile([128, 1], fp32)
            nc.scalar.square(junk2[:], junk[:])
            junk3 = constp.tile([128, 1], fp32)
            nc.scalar.sqrt(junk3[:], junk[:])

            # main matmuls + per-block reductions
            red = None  # [128, 24] used: dot j | ns 8+j | nd 16+j
            ps_t = [[None, None] for _ in range(4)]  # [cb][dtile]

            def reduce_cb(cb):
                nonlocal red
                if red is None:
                    # same tag as ps: cycles within the 8-buffer pool (the
                    # 9th allocation reuses a released phase-0 bank)
                    red = psum.tile([128, 512], fp32, name="red", tag="ps")
                tiles = []
                for dt in range(2):
                    ps = ps_t[cb][dt]
                    hs_sb = stage.tile(
                        [128, 256], fp32, name=f"hs_{cb}_{dt}", tag="hs"
                    )
                    nc.vector.tensor_tensor(
                        out=hs_sb[:], in0=ps[:, 0:256], in1=bias_sb[:, dt, :],
                        op=mybir.AluOpType.add,
                    )
                    hd_sb = stage.tile(
                        [128, 256], fp32, name=f"hd_{cb}_{dt}", tag="hd"
                    )
                    nc.vector.tensor_tensor(
                        out=hd_sb[:], in0=ps[:, 256:512], in1=bias_sb[:, dt, :],
                        op=mybir.AluOpType.add,
                    )
                    P = stage.tile([128, 256], bf16, name=f"P_{cb}_{dt}", tag="P")
                    nc.vector.tensor_tensor(
                        out=P[:], in0=hs_sb[:], in1=hd_sb[:],
                        op=mybir.AluOpType.mult,
                    )
                    S2 = stage.tile([128, 256], bf16, name=f"S2_{cb}_{dt}", tag="S2")
                    nc.scalar.square(S2[:], hs_sb[:])
                    D2 = stage.tile([128, 256], bf16, name=f"D2_{cb}_{dt}", tag="D2")
                    nc.scalar.square(D2[:], hd_sb[:])
                    tiles.append((P, S2, D2))
                # dt pair adjacent per accumulation group: a group-start in
                # the same PSUM bank between a pair would reset its state
                for c in range(2):
                    j = cb * 2 + c
                    for q in range(3):
                        for dt in range(2):
                            nc.tensor.matmul(
                                out=red[:, 8 * q + j:8 * q + j + 1],
                                lhsT=tiles[dt][q][:, c * 128:(c + 1) * 128],
                                rhs=ones1[:],
                                start=(dt == 0),
                                stop=(dt == 1),
                            )

            GROUPS = [[0, 1], [2], [3]]
            with nc.named_scope("matmul"):
                for gi, grp in enumerate(GROUPS):
                    for cb in grp:
                        for dt in range(2):
                            ps_t[cb][dt] = psum.tile(
                                [128, 512], fp32,
                                name=f"ps_{cb}_{dt}", tag="ps",
                            )
                    for t in range(KT2):
                        for dt in range(2):
                            for cb in grp:
                                nc.tensor.matmul(
                                    out=ps_t[cb][dt][:],
                                    lhsT=e_t[t][:, :, dt * 128:(dt + 1) * 128],
                                    rhs=a_t[cb][t][:],
                                    start=(t == 0),
                                    stop=(t == KT2 - 1),
                                    perf_mode=DR,
                                )
                    with nc.named_scope(f"reduce{gi}"):
                        for cb in grp:
                            reduce_cb(cb)

            with nc.named_scope("final"):
                red_sb = constp.tile([128, 24], fp32)
                nc.vector.tensor_copy(red_sb[:], red[:, 0:24])
                nsnd = constp.tile([128, JT], fp32)
                nc.vector.tensor_tensor(
                    out=nsnd[:], in0=red_sb[:, 8:16], in1=red_sb[:, 16:24],
                    op=mybir.AluOpType.mult,
                )
                st = constp.tile([128, JT], fp32)
                nc.scalar.sqrt(st[:], nsnd[:])
                rinv = constp.tile([128, JT], fp32)
                nc.vector.reciprocal(rinv[:], st[:])
                ad = constp.tile([128, JT], fp32)
                nc.vector.tensor_scalar(
                    out=ad[:].bitcast(mybir.dt.uint32),
                    in0=red_sb[:, 0:8].bitcast(mybir.dt.uint32),
                    scalar1=0x7FFFFFFF, scalar2=None,
                    op0=mybir.AluOpType.bitwise_and,
                )
                nc.vector.tensor_tensor(
                    out=out_sb[:], in0=ad[:], in1=rinv[:],
                    op=mybir.AluOpType.mult,
                )
                nc.sync.dma_start(out=out[:], in_=out_sb[:])

    nc.compile()
    return nc


def _get_nc():
    if "nc" not in _CACHE:
        _CACHE["nc"] = _build()
    return _CACHE["nc"]


def kernel(edges, A_s, emb, Ws, bs):
    global LAST_RESULTS
    from concourse.bass_utils import run_bass_kernel_spmd

    e4 = ml_dtypes.float8_e4m3fn
    A = np.asarray(A_s, dtype=np.float32)
    E = np.asarray(emb, dtype=np.float32)
    W = np.asarray(Ws, dtype=np.float32)
    b = np.asarray(bs, dtype=np.float32)
    ed = np.asarray(edges)

    M = W[0].T @ W[1].T @ W[2].T                      # [D, D]
    E2 = E @ M                                        # [N, D]
    b_eff = (b[0] @ W[1].T + b[1]) @ W[2].T + b[2]    # [D]
    s = E2_SCALE_TARGET / np.abs(E2).max()
    E2s = E2 * s
    bias_tot = (0.5 * E2.sum(axis=0) + b_eff) * s     # mean part + bias
    biasb = np.ascontiguousarray(
        np.broadcast_to(
            bias_tot.reshape(2, 128).T[:, :, None].astype(np.float32),
            (128, 2, D),
        )
    )

    # e28[p, t, ko, d] = E2q[t*256 + ko*128 + p, d]
    E2q = np.clip(E2s, -240.0, 240.0).astype(e4)
    e28 = np.ascontiguousarray(
        E2q.reshape(KT2, 2, 128, D).transpose(2, 0, 1, 3)
    )

    # residual of A in fp8 (mean folded into bias above)
    Rq = (A - 0.5).astype(e4)                         # [N, N]

    in_maps = []
    for c in range(N_CORES):
        e = ed[c * EPC:(c + 1) * EPC].astype(np.int64)
        src, dst = e[:, 0], e[:, 1]
        a8 = np.empty((128, 4, KT2, 2, 512), dtype=e4)
        for cb in range(4):
            sl = slice(cb * 256, (cb + 1) * 256)
            nodes = np.concatenate([src[sl], dst[sl]])  # [512]
            Rga = Rq[nodes]                             # [512, 8192] fp8
            # -> [ki, t, ko, col]
            a8[:, cb] = Rga.T.reshape(KT2, 2, 128, 512).transpose(2, 0, 1, 3)
        in_maps.append({"a8": a8, "e28": e28, "biasb": biasb})

    nc = _get_nc()
    kw = {}
    if os.environ.get("KERNEL_TRACE_KW"):
        import json
        kw = json.loads(os.environ["KERNEL_TRACE_KW"])
    res = run_bass_kernel_spmd(nc, in_maps, list(range(N_CORES)), **kw)
    LAST_RESULTS = res

    out = np.concatenate(
        [np.ascontiguousarray(res.results[c]["out"].T).reshape(-1)
         for c in range(N_CORES)]
    )
    return np.maximum(out, 0.0).astype(np.float32)
